# revision 1
# baseline (speedup 1.0000x reference)
"""Multi-head attention Trainium2 Bass kernel.

Problem: x:(4,512,1024), Wq/Wk/Wv/Wo:(512,512), H=8 heads, d=64.
  q = Wq@x ; k = Wk@x ; v = Wv@x  (per batch, 1x1 conv == channel matmul)
  per head: S[i,j] = q[:,i].k[:,j] ; attn = softmax_j(S) ; y = attn @ v
  out = Wo @ y

Sharding: 8 cores = (batch b, head-half g).  Core (b,g) handles batch b,
local heads g*4..g*4+3 and computes the partial output
out_p = Wo[:, g*256:(g+1)*256] @ y_g which the host sums pairwise.

Layout: scores are computed TRANSPOSED (S^T[j,i] = k^T q) so no PE
transposes are needed anywhere; softmax runs without max subtraction
(scores max ~52 < 88 overflow limit); the PV matmul's lhsT carries an
extra ones column so the softmax denominator falls out of the same
matmul; the rowsum row is replicated across partitions by a K=1
ones-row matmul and inverted with a fast approximate reciprocal.

All matmuls are float32r: 1 row/cycle at 2.4 GHz once the PE clock gate
(HAM) is warm, with fp32-level dynamic range and ~1e-3 accuracy.
"""

import numpy as np

import concourse.bass as bass
import concourse.tile as tile
from concourse import bacc
from concourse import mybir
from concourse.bass_utils import run_bass_kernel_spmd

F32 = mybir.dt.float32
F32R = mybir.dt.float32r
BF16 = mybir.dt.bfloat16

P = 128
C = 512          # channels
NSEQ = 1024      # sequence length
D = 64           # head dim
HL = 4           # local heads per core
KC = C // P      # 4 contraction tiles over channels
J = NSEQ // P    # 8 key tiles

_NC_CACHE = {}


def build_nc():
    nc = bacc.Bacc("TRN2")

    x = nc.dram_tensor("x", [C, NSEQ], F32R, kind="ExternalInput")
    wqkv = nc.dram_tensor("wqkv_t", [C, 3, 2 * P], F32R, kind="ExternalInput")
    wo = nc.dram_tensor("wo_t", [D, HL, C], F32R, kind="ExternalInput")
    out = nc.dram_tensor("out_p", [C, NSEQ], F32, kind="ExternalOutput")

    with tile.TileContext(nc) as tc:
        with (
            tc.tile_pool(name="consts", bufs=1) as consts,
            tc.tile_pool(name="epool", bufs=6) as epool,
            tc.tile_pool(name="ypool", bufs=6) as ypool,
            tc.tile_pool(name="rpool", bufs=4) as rpool,
            tc.tile_pool(name="opool", bufs=2) as opool,
            tc.tile_pool(name="pp", bufs=2, space="PSUM") as pp,
        ):
            # ---- load inputs: per-k-tile tiles so Tile's per-tile dep
            # tracking lets each projection matmul start as soon as ITS
            # chunk is in, not after the whole load.
            x_t = x.rearrange("(kc p) n -> p kc n", p=P)
            w_t = wqkv.rearrange("(kc p) w m -> p kc w m", p=P)
            x_sb, w3_sb = [], []
            for kc in range(KC):
                t = consts.tile([P, 3, 2 * P], F32R, tag=f"w{kc}")
                nc.sync.dma_start(t, w_t[:, kc])
                w3_sb.append(t)
                xt = consts.tile([P, NSEQ], F32R, tag=f"x{kc}")
                nc.sync.dma_start(xt, x_t[:, kc])
                x_sb.append(xt)
            wq_sb = [t[:, 0, :] for t in w3_sb]
            wk_sb = [t[:, 1, :] for t in w3_sb]
            wv_sb = [t[:, 2, :] for t in w3_sb]
            # dependency-free warm-up matmuls fill the PE during the load
            # window so the HAM clock gate reaches 8/8 before projections.
            warm_sb = consts.tile([P, 512], F32R)
            nc.vector.memset(warm_sb.bitcast(mybir.dt.uint32), 0)
            for wi in range(14):
                pw = pp.tile([P, 512], F32, tag="po", name="pw", bufs=2)
                nc.tensor.matmul(pw, lhsT=warm_sb[:, 0:P], rhs=warm_sb,
                                 start=True, stop=True)
            wot_sb = consts.tile([D, HL, C], F32R)
            nc.sync.dma_start(wot_sb, wo[:, :, :])

            # ---- q, k projections: (256,512)@(512,1024) ----
            q_sb = [consts.tile([P, NSEQ], F32R, tag=f"q{m}", name=f"q{m}")
                    for m in range(2)]
            k_sb = [consts.tile([P, NSEQ], F32R, tag=f"k{m}", name=f"k{m}")
                    for m in range(2)]
            tags = ["po", "py0", "py1"]
            ti = 0
            for w_sb, dst in ((wq_sb, q_sb), (wk_sb, k_sb)):
                for m in range(2):
                    for nn in range(2):
                        tag = tags[ti % 3]; ti += 1
                        ps = pp.tile([P, 512], F32, tag=tag, name=tag,
                                     bufs=2 if tag == "po" else 1)
                        for kc in range(KC):
                            nc.tensor.matmul(
                                ps,
                                lhsT=w_sb[kc][:, m * P:(m + 1) * P],
                                rhs=x_sb[kc][:, nn * 512:(nn + 1) * 512],
                                start=(kc == 0),
                                stop=(kc == KC - 1),
                            )
                        nc.vector.tensor_copy(
                            out=dst[m][:, nn * 512:(nn + 1) * 512], in_=ps
                        )

            # ---- v^T projection: out[j, d'] tiles, plus ones column ----
            vt_sb = [
                consts.tile([P, HL, D + 1], F32R, tag=f"vt{j}", name=f"vt{j}")
                for j in range(J)
            ]
            # memset can't target f32r; zero the ones-column via a uint32
            # view, then produce rounded-f32r 1.0s with ACT (0*x + 1).
            for j in range(J):
                ones_col = vt_sb[j][:, :, D:D + 1]
                nc.vector.memset(ones_col.bitcast(mybir.dt.uint32), 0)
                nc.scalar.activation(
                    out=ones_col, in_=ones_col,
                    func=mybir.ActivationFunctionType.Identity,
                    bias=1.0, scale=0.0,
                )
            for j in range(J):
                tag = tags[ti % 3]; ti += 1
                psv = pp.tile([P, 512], F32, tag=tag, name=tag,
                              bufs=2 if tag == "po" else 1)
                for kc in range(KC):
                    nc.tensor.matmul(
                        psv[:, 0:2 * P],
                        lhsT=x_sb[kc][:, j * P:(j + 1) * P],
                        rhs=wv_sb[kc],
                        start=(kc == 0),
                        stop=(kc == KC - 1),
                    )
                nc.vector.tensor_copy(
                    out=vt_sb[j][:, :, 0:D],
                    in_=psv[:, 0:2 * P].rearrange("p (h d) -> p h d", h=HL),
                )

            ones64 = consts.tile([P, D], F32R)
            nc.vector.memset(ones64.bitcast(mybir.dt.uint32), 0)
            nc.scalar.activation(
                out=ones64[D:D + 1, :], in_=ones64[D:D + 1, :],
                func=mybir.ActivationFunctionType.Identity,
                bias=1.0, scale=0.0,
            )

            # ---- attention: head pairs on alternating PE row groups,
            # i processed in halves; outproj per i-half overlaps the next
            # half's attention.
            y_sb = [
                consts.tile([D, NSEQ], F32R, tag=f"y{h}", name=f"y{h}")
                for h in range(HL)
            ]
            out_t = out.rearrange("(m p) n -> p m n", p=P)
            pending = []

            def emit_pending():
                for fn in pending:
                    fn()
                pending.clear()

            for ihalf in range(2):
                isl = slice(ihalf * 512, (ihalf + 1) * 512)
                for pair in ((0, 1) if ihalf == 0 else (1, 0)):
                    py = [
                        pp.tile([P, 512], F32, tag="py0", name="py0", bufs=1),
                        pp.tile([P, 512], F32, tag="py1", name="py1", bufs=1),
                    ]
                    for j in range(J):
                        # both lanes' scores into one 2-bank psum tile so a
                        # single 1024-wide exp serves the head pair
                        ps = pp.tile([P, 1024], F32, tag="s", name="s",
                                     bufs=2)
                        for lane in range(2):
                            hp = lane * D
                            nc.tensor.matmul(
                                ps[:, lane * 512:(lane + 1) * 512],
                                lhsT=k_sb[pair][hp:hp + D, j * P:(j + 1) * P],
                                rhs=q_sb[pair][hp:hp + D, isl],
                                start=True, stop=True,
                            )
                        e = epool.tile([P, 1024], F32R, tag="e", name="e")
                        nc.scalar.activation(
                            out=e, in_=ps,
                            func=mybir.ActivationFunctionType.Exp,
                        )
                        for lane in range(2):
                            nc.tensor.matmul(
                                py[lane][0:D + 1, :],
                                lhsT=vt_sb[j][:, 2 * pair + lane, :],
                                rhs=e[:, lane * 512:(lane + 1) * 512],
                                start=(j == 0), stop=(j == J - 1),
                            )
                    # the 65-row copies release the PSUM accumulators now;
                    # the replicate/1-over-x/scale tail is emitted one block
                    # later so its PE work lands in scheduling slack instead
                    # of pinching the next block's first scores.
                    emit_pending()
                    yus = []
                    for lane in range(2):
                        yu = ypool.tile([D + 1, 512], F32R, tag="yu",
                                        name="yu")
                        nc.vector.tensor_copy(out=yu, in_=py[lane][0:D + 1, :])
                        yus.append(yu)

                    def drain(yus=yus, pair=pair, isl=isl):
                        for lane in range(2):
                            h = 2 * pair + lane
                            pr = pp.tile([D, 512], F32, tag=f"py{lane}",
                                         name="pr", bufs=1)
                            nc.tensor.matmul(
                                pr, lhsT=ones64[D:D + 1, :],
                                rhs=yus[lane][D:D + 1, :],
                                start=True, stop=True,
                            )
                            rr = rpool.tile([D, 512], F32, tag="rr",
                                            name="rr")
                            nc.vector.reciprocal_approx_fast(out=rr, in_=pr)
                            nc.vector.tensor_tensor(
                                out=y_sb[h][:, isl],
                                in0=yus[lane][0:D, :], in1=rr,
                                op=mybir.AluOpType.mult,
                            )

                    drain()
                emit_pending()
                # output projection for this i-half (hoists into the next
                # half's attention once the 4 y tiles are ready)
                horder = (0, 1, 2, 3) if ihalf == 0 else (2, 3, 0, 1)
                for m in range(4):
                    po = pp.tile([P, 512], F32, tag="po", name="po", bufs=2)
                    for hi, h in enumerate(horder):
                        nc.tensor.matmul(
                            po,
                            lhsT=wot_sb[:, h, m * P:(m + 1) * P],
                            rhs=y_sb[h][:, isl],
                            start=(hi == 0),
                            stop=(hi == HL - 1),
                        )
                    ot = opool.tile([P, 512], F32, tag="ot")
                    nc.vector.tensor_copy(out=ot, in_=po)
                    nc.sync.dma_start(out=out_t[:, m, isl], in_=ot)

    nc.compile()
    return nc


def get_nc():
    if "nc" not in _NC_CACHE:
        _NC_CACHE["nc"] = build_nc()
    return _NC_CACHE["nc"]


def make_in_maps(x, Wq, Wk, Wv, Wo):
    in_maps = []
    for core in range(8):
        b, g = core // 2, core % 2
        sl = slice(g * 256, (g + 1) * 256)
        wqkv = np.stack(
            [Wq[sl, :].T, Wk[sl, :].T, Wv[sl, :].T], axis=1
        )  # (512, 3, 256)
        in_maps.append({
            "x": np.ascontiguousarray(x[b]),
            "wqkv_t": np.ascontiguousarray(wqkv),
            # [d, h, o] so lhsT slices are contiguous per head
            "wo_t": np.ascontiguousarray(
                Wo[:, sl].reshape(C, HL, D).transpose(2, 1, 0)
            ),
        })
    return in_maps


LAST_RESULTS = {}


def kernel(x, Wq, Wk, Wv, Wo, _trace=False):
    x = np.asarray(x, dtype=np.float32)
    Wq = np.asarray(Wq, dtype=np.float32)
    Wk = np.asarray(Wk, dtype=np.float32)
    Wv = np.asarray(Wv, dtype=np.float32)
    Wo = np.asarray(Wo, dtype=np.float32)

    nc = get_nc()
    in_maps = make_in_maps(x, Wq, Wk, Wv, Wo)
    res = run_bass_kernel_spmd(
        nc, in_maps, core_ids=list(range(8)), trace=_trace
    )
    LAST_RESULTS["res"] = res
    parts = [np.asarray(r["out_p"]) for r in res.results]
    out = np.stack([parts[2 * b] + parts[2 * b + 1] for b in range(4)])
    return out



# revision 7
# speedup vs baseline: 1.0756x; 1.0756x over previous
"""Multi-head attention Trainium2 Bass kernel (v2, bf16).

Problem: x:(4,512,1024), Wq/Wk/Wv/Wo:(512,512), H=8 heads, d=64.
  q = Wq@x ; k = Wk@x ; v = Wv@x  (1x1 conv == channel matmul)
  per head: S[i,j] = q[:,i].k[:,j] ; attn = softmax_j(S) ; y = attn @ v
  out = Wo @ y

Sharding: 8 cores = (batch b, head-half g).  Core (b,g) handles batch b,
local heads g*4..g*4+3 and computes the partial output
out_p = Wo[:, g*256:(g+1)*256] @ y_g which the host sums pairwise.

Design notes (v2):
- All operands bf16 (host-cast): halves DMA, 1 row/cycle on the PE like
  f32r, f32 PSUM accumulation throughout.
- Scores are computed transposed (S^T = k^T q) per head pair with the
  two head lanes ROW-TILED onto PE row groups 0-63 / 64-127 so both
  64-contraction matmuls can run concurrently (full array).
- exp on the ACT engine is the hard floor (32 x [128,1024] ~ 36us);
  the emission order keeps ACT dense: the four (pair, i-half) attention
  blocks interleave V-projection (A), the second QK projection (B) and
  the i0 output projection (D) under the exp stream so the PE never
  idles long (HAM stays at 8/8).
- PV keeps the ones-column trick (M=65): row 64 of the PV accumulator
  is the softmax denominator. The denominator is broadcast across 64
  partitions with two tiny K=1 col-tiled matmuls whose rhs is row 64 of
  the yu tile directly; reciprocal on DVE; the normalize multiplies run
  on GPSIMD/DVE (all-SBUF) and write a pair-stacked y layout [128, n]
  (head 2g on partitions 0-63, head 2g+1 on 64-127).
- Output projection is row-tiled over the stacked y pairs: two
  concurrent K=64 matmuls per (m, g) into two PSUM banks, summed by the
  DVE during the PSUM->SBUF drain copy.
"""

import numpy as np
import ml_dtypes

import concourse.bass as bass
import concourse.tile as tile
from concourse import bacc
from concourse import mybir
from concourse.bass_utils import run_bass_kernel_spmd

F32 = mybir.dt.float32
BF16 = mybir.dt.bfloat16
NPBF16 = ml_dtypes.bfloat16

P = 128
C = 512          # channels
NSEQ = 1024      # sequence length
D = 64           # head dim
HL = 4           # local heads per core
KC = C // P      # 4 contraction tiles over channels
J = NSEQ // P    # 8 key tiles

_NC_CACHE = {}


def build_nc():
    nc = bacc.Bacc("TRN2")

    x = nc.dram_tensor("x", [C, NSEQ], BF16, kind="ExternalInput")
    wqkv = nc.dram_tensor("wqkv_t", [C, 3, 2 * P], BF16, kind="ExternalInput")
    # paired Wo: [d + 64*(h%2), h//2, c_out]
    wo = nc.dram_tensor("wo_p", [P, 2, C], BF16, kind="ExternalInput")
    out = nc.dram_tensor("out_p", [C, NSEQ], BF16, kind="ExternalOutput")

    with tile.TileContext(nc) as tc:
        with (
            tc.tile_pool(name="consts", bufs=1) as consts,
            tc.tile_pool(name="epool", bufs=4) as epool,
            tc.tile_pool(name="ypool", bufs=3) as ypool,
            tc.tile_pool(name="rpool", bufs=2) as rpool,
            tc.tile_pool(name="opool", bufs=2) as opool,
            tc.tile_pool(name="pp", bufs=2, space="PSUM") as pp,
        ):
            # ---- persistent tiles
            warm = consts.tile([P, 512], BF16)
            ones_sel = consts.tile([P, D], BF16)
            w_sb = consts.tile([P, KC, 3, 2 * P], BF16)
            x_sb = [consts.tile([P, NSEQ], BF16, tag=f"x{kc}", name=f"x{kc}")
                    for kc in range(KC)]
            wot_sb = consts.tile([P, 2, C], BF16)
            q_sb = [consts.tile([P, NSEQ], BF16, name=f"q{m}")
                    for m in range(2)]
            k_sb = [consts.tile([P, NSEQ], BF16, name=f"k{m}")
                    for m in range(2)]
            vt_sb = [consts.tile([P, HL, D + 1], BF16, name=f"vt{j}")
                     for j in range(J)]
            # pair-stacked y: yt[g][64*lane + d, n] = y_{head 2g+lane}[d, n]
            yt = [consts.tile([P, NSEQ], BF16, name=f"yt{g}")
                  for g in range(2)]

            # ---- init constants (gpsimd; keeps DVE/ACT free)
            nc.gpsimd.memset(warm, 0)
            nc.gpsimd.memset(ones_sel, 1.0)
            for j in range(J):
                nc.gpsimd.memset(vt_sb[j][:, :, D:D + 1], 1.0)

            def po_tile(name="po"):
                return pp.tile([P, 512], F32, tag="po", name=name, bufs=2)

            # ---- warm-up matmuls release the HAM clock gate during load
            for wi in range(8):
                nc.tensor.matmul(po_tile("pw"), lhsT=warm[:, 0:P], rhs=warm,
                                 start=True, stop=True)

            # ---- input DMAs: w first, then x by (seq-half, kc) so the
            # first QK projection group can start earliest.
            nc.sync.dma_start(w_sb, wqkv.rearrange("(kc p) w m -> p kc w m",
                                                   p=P))
            x_t = x.rearrange("(kc p) (sh n) -> p kc sh n", p=P, n=512)
            for sh in range(2):
                for kc in range(KC):
                    nc.sync.dma_start(
                        x_sb[kc][:, sh * 512:(sh + 1) * 512], x_t[:, kc, sh])
            nc.sync.dma_start(wot_sb, wo[:, :, :])

            # ---- QK projection pair0 (m=0), seq-half 0 first
            def proj_group(dst, w_idx, m, nn):
                ps = po_tile()
                for kc in range(KC):
                    nc.tensor.matmul(
                        ps,
                        lhsT=w_sb[:, kc, w_idx, m * P:(m + 1) * P],
                        rhs=x_sb[kc][:, nn * 512:(nn + 1) * 512],
                        start=(kc == 0),
                        stop=(kc == KC - 1),
                    )
                nc.vector.tensor_copy(
                    out=dst[:, nn * 512:(nn + 1) * 512], in_=ps)

            proj_group(q_sb[0], 0, 0, 0)
            proj_group(k_sb[0], 1, 0, 0)
            proj_group(k_sb[0], 1, 0, 1)
            proj_group(q_sb[0], 0, 0, 1)

            def vproj(j):
                psv = po_tile()
                for kc in range(KC):
                    nc.tensor.matmul(
                        psv[:, 0:2 * P],
                        lhsT=x_sb[kc][:, j * P:(j + 1) * P],
                        rhs=w_sb[:, kc, 2, :],
                        start=(kc == 0),
                        stop=(kc == KC - 1),
                    )
                nc.vector.tensor_copy(
                    out=vt_sb[j][:, :, 0:D],
                    in_=psv[:, 0:2 * P].rearrange("p (h d) -> p h d", h=HL),
                )

            # QK m=1, split into 8 steps of 2 matmuls; order gets
            # k1/q1 seq-half0 done first (block C consumes those first).
            _qk_ps = {}
            _QK_ORDER = [(0, 0), (1, 0), (1, 1), (0, 1)]  # (w_idx, nn)

            def qk_m1_step(step):
                w_idx, nn = _QK_ORDER[step // 2]
                kh = step % 2
                if kh == 0:
                    _qk_ps[(w_idx, nn)] = po_tile()
                ps = _qk_ps[(w_idx, nn)]
                for kc in (2 * kh, 2 * kh + 1):
                    nc.tensor.matmul(
                        ps,
                        lhsT=w_sb[:, kc, w_idx, P:2 * P],
                        rhs=x_sb[kc][:, nn * 512:(nn + 1) * 512],
                        start=(kc == 0),
                        stop=(kc == KC - 1),
                    )
                if kh == 1:
                    dst = q_sb[1] if w_idx == 0 else k_sb[1]
                    nc.vector.tensor_copy(
                        out=dst[:, nn * 512:(nn + 1) * 512], in_=ps)

            def attention_block(pair, ihalf, interleave=None, steps=0):
                """j-loop for one (pair, i-half); interleave(step) emits
                other PE work between the per-j matmul groups."""
                isl = slice(ihalf * 512, (ihalf + 1) * 512)
                py = [
                    pp.tile([P, 512], F32, tag="py0", name="py0", bufs=1),
                    pp.tile([P, 512], F32, tag="py1", name="py1", bufs=1),
                ]
                step = 0
                for j in range(J):
                    ps = pp.tile([P, 1024], F32, tag="s", name="s", bufs=2)
                    for lane in range(2):
                        hp = lane * D
                        nc.tensor.matmul(
                            ps[:, lane * 512:(lane + 1) * 512],
                            lhsT=k_sb[pair][hp:hp + D, j * P:(j + 1) * P],
                            rhs=q_sb[pair][hp:hp + D, isl],
                            start=True, stop=True,
                        )
                    e = epool.tile([P, 1024], BF16, tag="e", name="e")
                    nc.scalar.activation(
                        out=e, in_=ps,
                        func=mybir.ActivationFunctionType.Exp,
                    )
                    for lane in range(2):
                        nc.tensor.matmul(
                            py[lane][0:D + 1, :],
                            lhsT=vt_sb[j][:, 2 * pair + lane, :],
                            rhs=e[:, lane * 512:(lane + 1) * 512],
                            start=(j == 0), stop=(j == J - 1),
                        )
                    while step < (steps * (j + 1) + J - 1) // J:
                        interleave(step)
                        step += 1
                while step < steps:
                    interleave(step)
                    step += 1
                return py

            def drain(py, pair, ihalf):
                isl = slice(ihalf * 512, (ihalf + 1) * 512)
                # lane0: y rows + denominator row in one copy (base 0)
                yu0 = ypool.tile([D + 1, 512], BF16, tag="yu", name="yu")
                nc.vector.tensor_copy(out=yu0, in_=py[0][0:D + 1, :])
                # lane1: y rows shifted to the upper partition half so the
                # normalize + stacked out-projection stay base-aligned
                yu1 = ypool.tile([P, 512], BF16, tag="yu1", name="yu1")
                nc.vector.tensor_copy(out=yu1[D:2 * D, :],
                                      in_=py[1][0:D, :])
                dn1 = rpool.tile([D + 1, 512], BF16, tag="dn", name="dn")
                nc.vector.tensor_copy(out=dn1[D:D + 1, :],
                                      in_=py[1][D:D + 1, :])
                # denominator broadcast: two tiny K=1 col-tiled matmuls
                pr = po_tile("pr")
                nc.tensor.matmul(
                    pr[0:D, :], lhsT=ones_sel[D:D + 1, :],
                    rhs=yu0[D:D + 1, :], start=True, stop=True,
                )
                nc.tensor.matmul(
                    pr[D:2 * D, :], lhsT=ones_sel[D:D + 1, :],
                    rhs=dn1[D:D + 1, :], start=True, stop=True,
                )
                rr = rpool.tile([P, 512], F32, tag="rr", name="rr")
                nc.vector.reciprocal_approx_fast(out=rr, in_=pr)
                # normalize into the pair-stacked y tile (all-SBUF, gpsimd)
                nc.gpsimd.tensor_tensor(
                    out=yt[pair][0:D, isl],
                    in0=yu0[0:D, :], in1=rr[0:D, :],
                    op=mybir.AluOpType.mult,
                )
                nc.gpsimd.tensor_tensor(
                    out=yt[pair][D:2 * D, isl],
                    in0=yu1[D:2 * D, :], in1=rr[D:2 * D, :],
                    op=mybir.AluOpType.mult,
                )

            def outproj_mm(m, g, po, isl):
                # stacked K=128 contraction sums the head pair directly
                nc.tensor.matmul(
                    po,
                    lhsT=wot_sb[:, g, m * P:(m + 1) * P],
                    rhs=yt[g][:, isl],
                    start=(g == 0), stop=(g == 1),
                )

            # ---- block A: pair0 / i-half0, V projection just-in-time
            vproj(0)
            vproj(1)
            pyA = attention_block(
                0, 0, interleave=lambda s: vproj(s + 2), steps=6)
            drain(pyA, 0, 0)

            # ---- block B: pair0 / i-half1, QK m=1 interleaved
            pyB = attention_block(0, 1, interleave=qk_m1_step, steps=8)
            drain(pyB, 0, 1)

            # ---- block C: pair1 / i-half0 (PE-light)
            pyC = attention_block(1, 0)
            drain(pyC, 1, 0)

            # ---- block D: pair1 / i-half1 with i-half0 out-projection
            # interleaved (m-sequential: 2 matmuls + 1 copy per m)
            ot0 = opool.tile([P, 4, 512], BF16, tag="ot")
            _d_state = {}

            def outproj_i0_step(step):
                m, phase = divmod(step, 3)
                if phase == 0:
                    _d_state[m] = po_tile()
                    outproj_mm(m, 0, _d_state[m], slice(0, 512))
                elif phase == 1:
                    outproj_mm(m, 1, _d_state[m], slice(0, 512))
                else:
                    nc.vector.tensor_copy(out=ot0[:, m, :], in_=_d_state[m])

            pyD = attention_block(1, 1, interleave=outproj_i0_step, steps=12)
            out_t = out.rearrange("(m p) n -> p m n", p=P)
            nc.sync.dma_start(out=out_t[:, :, 0:512], in_=ot0)
            drain(pyD, 1, 1)

            # ---- tail: out-projection i-half1
            ot1 = opool.tile([P, 4, 512], BF16, tag="ot")
            for m in range(4):
                po = po_tile()
                outproj_mm(m, 0, po, slice(512, 1024))
                outproj_mm(m, 1, po, slice(512, 1024))
                nc.vector.tensor_copy(out=ot1[:, m, :], in_=po)
            nc.sync.dma_start(out=out_t[:, :, 512:1024], in_=ot1)

    nc.compile()
    return nc


def get_nc():
    if "nc" not in _NC_CACHE:
        _NC_CACHE["nc"] = build_nc()
    return _NC_CACHE["nc"]


def make_in_maps(x, Wq, Wk, Wv, Wo):
    in_maps = []
    for core in range(8):
        b, g = core // 2, core % 2
        sl = slice(g * 256, (g + 1) * 256)
        wqkv = np.stack(
            [Wq[sl, :].T, Wk[sl, :].T, Wv[sl, :].T], axis=1
        )  # (512, 3, 256)
        # paired Wo layout: wo_p[d + 64*(h%2), h//2, c_out]
        wo_l = Wo[:, sl].reshape(C, HL, D)          # (c_out, h, d)
        wo_p = np.empty((P, 2, C), dtype=np.float32)
        for h in range(HL):
            wo_p[(h % 2) * D:(h % 2) * D + D, h // 2, :] = wo_l[:, h, :].T
        in_maps.append({
            "x": np.ascontiguousarray(x[b]).astype(NPBF16),
            "wqkv_t": np.ascontiguousarray(wqkv).astype(NPBF16),
            "wo_p": np.ascontiguousarray(wo_p).astype(NPBF16),
        })
    return in_maps


LAST_RESULTS = {}


def kernel(x, Wq, Wk, Wv, Wo, _trace=False):
    x = np.asarray(x, dtype=np.float32)
    Wq = np.asarray(Wq, dtype=np.float32)
    Wk = np.asarray(Wk, dtype=np.float32)
    Wv = np.asarray(Wv, dtype=np.float32)
    Wo = np.asarray(Wo, dtype=np.float32)

    nc = get_nc()
    in_maps = make_in_maps(x, Wq, Wk, Wv, Wo)
    res = run_bass_kernel_spmd(
        nc, in_maps, core_ids=list(range(8)), trace=_trace
    )
    LAST_RESULTS["res"] = res
    parts = [np.asarray(r["out_p"]).astype(np.float32) for r in res.results]
    out = np.stack([parts[2 * b] + parts[2 * b + 1] for b in range(4)])
    return out


# revision 13
# speedup vs baseline: 1.1894x; 1.1059x over previous
"""Multi-head attention Trainium2 Bass kernel (v3, fp16/f32r).

Problem: x:(4,512,1024), Wq/Wk/Wv/Wo:(512,512), H=8 heads, d=64.
  q = Wq@x ; k = Wk@x ; v = Wv@x  (1x1 conv == channel matmul)
  per head: S[i,j] = q[:,i].k[:,j] ; attn = softmax_j(S) ; y = attn @ v
  out = Wo @ y

Sharding: 8 cores = (batch b, head-half g).  Core (b,g) handles batch b,
local heads g*4..g*4+3 and computes the partial output
out_p = Wo[:, g*256:(g+1)*256] @ y_g which the host sums pairwise.

Design notes (v3):
- Inputs/projections in fp16 (host-cast): half the DMA of f32, 1
  row/cycle on the PE, and an 11-bit mantissa so score differences stay
  accurate (bf16 cost 1.3e-2 rel err; fp16 ~1e-3).  exp output e and v
  stay f32r (range to e^52 and ACT writes f32 faster than bf16).
- Scores are computed transposed (S^T = k^T q) per head pair with the
  two head lanes ROW-TILED onto PE row groups 0-63 / 64-127 (auto
  tile_position) so both K=64 matmuls run concurrently.
- The ACT-engine exp stream (32 x [128,1024] ~ 36us) is the floor; the
  emission order keeps it dense: block A (pair0/i0) interleaves the
  just-in-time V projection and the pair0 nn=1 QK groups, block B
  (pair0/i1) the pair1 QK projection, block D (pair1/i1) the i0 output
  projection.  Each block's softmax-denominator/normalize chain is
  deferred into the next block so it never stalls the PE FIFO.
- PV keeps the ones-column trick (M=65): row 64 of the PV accumulator
  is the denominator.  It is broadcast across partitions by two K=1
  col-tiled matmuls (rhs = the denominator rows in SBUF), reciprocal on
  DVE, normalize on GPSIMD into a pair-stacked y layout [128, n]
  (head 2g on partitions 0-63, head 2g+1 on 64-127, via a partition-
  shifting PSUM->SBUF copy).
- With y pair-stacked, the output projection is a plain K=128 matmul
  per (m, g): the contraction sums the head pair directly.
"""

import numpy as np

import concourse.bass as bass
import concourse.tile as tile
from concourse import bacc
from concourse import mybir
from concourse.bass_utils import run_bass_kernel_spmd

F32 = mybir.dt.float32
F32R = mybir.dt.float32r
FP16 = mybir.dt.float16

P = 128
C = 512          # channels
NSEQ = 1024      # sequence length
D = 64           # head dim
HL = 4           # local heads per core
KC = C // P      # 4 contraction tiles over channels
J = NSEQ // P    # 8 key tiles

_NC_CACHE = {}


def build_nc():
    nc = bacc.Bacc("TRN2")

    x = nc.dram_tensor("x", [C, NSEQ], FP16, kind="ExternalInput")
    wqkv = nc.dram_tensor("wqkv_t", [C, 3, 2 * P], FP16, kind="ExternalInput")
    # paired Wo: [d + 64*(h%2), h//2, c_out]
    wo = nc.dram_tensor("wo_p", [P, 2, C], FP16, kind="ExternalInput")
    out = nc.dram_tensor("out_p", [C, NSEQ], FP16, kind="ExternalOutput")

    with tile.TileContext(nc) as tc:
        with (
            tc.tile_pool(name="consts", bufs=1) as consts,
            tc.tile_pool(name="epool", bufs=4) as epool,
            tc.tile_pool(name="ypool", bufs=3) as ypool,
            tc.tile_pool(name="rpool", bufs=2) as rpool,
            tc.tile_pool(name="opool", bufs=2) as opool,
            tc.tile_pool(name="pp", bufs=2, space="PSUM") as pp,
        ):
            # ---- persistent tiles
            warm = consts.tile([P, 512], FP16)
            # selector for the denominator broadcast: a K=33 matmul with
            # lhsT rows 0/32 routes lane0's denom to partitions 0-63 and
            # lane1's to 64-127 in one (0,0)-positioned matmul.
            sel = consts.tile([33, P], F32R)
            dd = consts.tile([33, 512], F32R)
            w_sb = [consts.tile([P, 3, 2 * P], FP16, name=f"w{kc}")
                    for kc in range(KC)]
            x_sb = [[consts.tile([P, 512], FP16, name=f"x{kc}_{sh}")
                     for sh in range(2)] for kc in range(KC)]
            wot_sb = consts.tile([P, 2, C], FP16)
            q_sb = [[consts.tile([P, 512], FP16, name=f"q{m}_{sh}")
                     for sh in range(2)] for m in range(2)]
            k_sb = [[consts.tile([P, 512], FP16, name=f"k{m}_{sh}")
                     for sh in range(2)] for m in range(2)]
            vt_sb = [consts.tile([P, HL, D + 1], F32R, name=f"vt{j}")
                     for j in range(J)]
            # pair-stacked y: yt[g][64*lane + d, n] = y_{head 2g+lane}[d, n]
            yt = [consts.tile([P, NSEQ], FP16, name=f"yt{g}")
                  for g in range(2)]

            # ---- init constants (gpsimd; keeps DVE/ACT free)
            nc.gpsimd.memset(warm, 0)
            # memset can't target f32r; write the f32 bit pattern of 1.0
            # through a uint32 view instead.
            ONE_F32 = 0x3F800000
            nc.gpsimd.memset(sel.bitcast(mybir.dt.uint32), 0)
            nc.gpsimd.memset(sel[0:1, 0:D].bitcast(mybir.dt.uint32), ONE_F32)
            nc.gpsimd.memset(sel[32:33, D:2 * D].bitcast(mybir.dt.uint32),
                             ONE_F32)
            # rows 1-31 of dd are never written; zero them once so the
            # 0-weight selector rows can't hit NaNs in the K=33 matmul.
            nc.gpsimd.memset(dd.bitcast(mybir.dt.uint32), 0)
            for j in range(J):
                nc.gpsimd.memset(
                    vt_sb[j][:, :, D:D + 1].bitcast(mybir.dt.uint32), ONE_F32)

            def po_tile(name="po", tag="po"):
                return pp.tile([P, 512], F32, tag=tag, name=name, bufs=2)

            # ---- warm-up matmuls release the HAM clock gate during load
            for wi in range(3):
                nc.tensor.matmul(po_tile("pw"), lhsT=warm[:, 0:P], rhs=warm,
                                 start=True, stop=True)

            # ---- input DMAs, alternating dispatch engines (SP/ACT) so
            # descriptor writes don't serialize; order = first-needed.
            w_t = wqkv.rearrange("(kc p) w m -> p kc w m", p=P)
            x_t = x.rearrange("(kc p) (sh n) -> p kc sh n", p=P, n=512)
            loads = []
            for kc in range(KC):
                loads.append((w_sb[kc], w_t[:, kc]))
            for sh in range(2):
                for kc in range(KC):
                    loads.append((x_sb[kc][sh], x_t[:, kc, sh]))
            loads.append((wot_sb, wo[:, :, :]))
            for i, (dst, src) in enumerate(loads):
                eng = nc.sync if i % 2 == 0 else nc.scalar
                eng.dma_start(dst, src)

            # ---- QK projections: group = (w_idx, m, nn), 4 matmuls
            def proj_group(w_idx, m, nn, split=None):
                """split=None: all 4 kc matmuls + copy. split=0: first 2.
                split=1: last 2 + copy."""
                key = (w_idx, m, nn)
                if split in (None, 0):
                    _proj_ps[key] = po_tile()
                ps = _proj_ps[key]
                kcs = range(KC) if split is None else (
                    (0, 1) if split == 0 else (2, 3))
                for kc in kcs:
                    nc.tensor.matmul(
                        ps,
                        lhsT=w_sb[kc][:, w_idx, m * P:(m + 1) * P],
                        rhs=x_sb[kc][nn],
                        start=(kc == 0),
                        stop=(kc == KC - 1),
                    )
                if split in (None, 1):
                    dst = (q_sb if w_idx == 0 else k_sb)[m][nn]
                    nc.vector.tensor_copy(out=dst, in_=ps)

            _proj_ps = {}
            proj_group(0, 0, 0)     # q pair0, seq-half 0
            proj_group(1, 0, 0)     # k pair0, seq-half 0

            def vproj(j):
                psv = po_tile()
                for kc in range(KC):
                    nc.tensor.matmul(
                        psv[:, 0:2 * P],
                        lhsT=x_sb[kc][j // 4][:, (j % 4) * P:(j % 4 + 1) * P],
                        rhs=w_sb[kc][:, 2, :],
                        start=(kc == 0),
                        stop=(kc == KC - 1),
                    )
                nc.vector.tensor_copy(
                    out=vt_sb[j][:, :, 0:D],
                    in_=psv[:, 0:2 * P].rearrange("p (h d) -> p h d", h=HL),
                )

            def attention_block(pair, ihalf, sched=None, first_block=False):
                """j-loop for one (pair, i-half); sched[j] lists callables
                emitted between exp_j and PV_j (fills the exp latency)."""
                sched = sched or {}
                py = [
                    pp.tile([P, 512], F32, tag="py0", name="py0", bufs=1),
                    pp.tile([P, 512], F32, tag="py1", name="py1", bufs=1),
                ]
                for j in range(J):
                    ps = pp.tile([P, 1024], F32, tag="s", name="s", bufs=2)
                    for lane in range(2):
                        hp = lane * D
                        nc.tensor.matmul(
                            ps[:, lane * 512:(lane + 1) * 512],
                            lhsT=k_sb[pair][j // 4][hp:hp + D,
                                                    (j % 4) * P:(j % 4 + 1) * P],
                            rhs=q_sb[pair][ihalf][hp:hp + D, :],
                            start=True, stop=True,
                        )
                    e = epool.tile([P, 1024], F32R, tag="e", name="e")
                    nc.scalar.activation(
                        out=e, in_=ps,
                        func=mybir.ActivationFunctionType.Exp,
                    )
                    # PE work emitted here fills the exp_j latency so the
                    # in-order PE queue doesn't stall at PV_j.
                    if first_block:
                        # V projection just-in-time, >=1 iteration ahead
                        if j == 0:
                            vproj(0)
                            vproj(1)
                        if j <= 5:
                            vproj(j + 2)
                    for fn in sched.get(j, ()):
                        fn()
                    for lane in range(2):
                        nc.tensor.matmul(
                            py[lane][0:D + 1, :],
                            lhsT=vt_sb[j][:, 2 * pair + lane, :],
                            rhs=e[:, lane * 512:(lane + 1) * 512],
                            start=(j == 0), stop=(j == J - 1),
                        )
                return py

            def drain_copies(py):
                # lane0: y rows at base 0
                yu0 = ypool.tile([D, 512], F32R, tag="yu", name="yu")
                nc.vector.tensor_copy(out=yu0, in_=py[0][0:D, :])
                # lane1: y rows shifted to the upper partition half so the
                # normalize + stacked out-projection stay base-aligned
                yu1 = ypool.tile([P, 512], F32R, tag="yu1", name="yu1")
                nc.vector.tensor_copy(out=yu1[D:2 * D, :],
                                      in_=py[1][0:D, :])
                # denominator rows into the shared selector rhs tile
                nc.vector.tensor_copy(out=dd[0:1, :], in_=py[0][D:D + 1, :])
                nc.vector.tensor_copy(out=dd[32:33, :],
                                      in_=py[1][D:D + 1, :])
                return (yu0, yu1)

            def drain_norm(state, pair, ihalf, pr_tag="po"):
                yu0, yu1 = state
                isl = slice(ihalf * 512, (ihalf + 1) * 512)
                # denominator broadcast: one K=33 selector matmul
                pr = pp.tile([P, 512], F32, tag=pr_tag, name="pr",
                             bufs=2 if pr_tag == "po" else 1)
                nc.tensor.matmul(
                    pr, lhsT=sel, rhs=dd, start=True, stop=True,
                )
                rr = rpool.tile([P, 512], F32, tag="rr", name="rr")
                nc.vector.reciprocal_approx_fast(out=rr, in_=pr)
                # normalize into the pair-stacked y tile (all-SBUF, gpsimd)
                nc.gpsimd.tensor_tensor(
                    out=yt[pair][0:D, isl],
                    in0=yu0, in1=rr[0:D, :],
                    op=mybir.AluOpType.mult,
                )
                nc.gpsimd.tensor_tensor(
                    out=yt[pair][D:2 * D, isl],
                    in0=yu1[D:2 * D, :], in1=rr[D:2 * D, :],
                    op=mybir.AluOpType.mult,
                )

            def outproj_mm(m, g, po, isl):
                # stacked K=128 contraction sums the head pair directly
                nc.tensor.matmul(
                    po,
                    lhsT=wot_sb[:, g, m * P:(m + 1) * P],
                    rhs=yt[g][:, isl],
                    start=(g == 0), stop=(g == 1),
                )

            # ---- block A: pair0/i0; V proj JIT + pair0 nn=1 QK groups
            def qk(w_idx, m, nn, split):
                return lambda: proj_group(w_idx, m, nn, split=split)

            pyA = attention_block(0, 0, first_block=True, sched={
                1: [qk(1, 0, 1, 0)], 3: [qk(1, 0, 1, 1)],
                4: [qk(0, 0, 1, 0)], 5: [qk(0, 0, 1, 1)],
            })
            stA = drain_copies(pyA)

            # ---- block B: pair0/i1; pair1 QK projection interleaved
            pyB = attention_block(0, 1, sched={
                0: [lambda: drain_norm(stA, 0, 0), qk(1, 1, 0, 0)],
                1: [qk(1, 1, 0, 1)], 2: [qk(0, 1, 0, 0)],
                3: [qk(0, 1, 0, 1)], 4: [qk(1, 1, 1, 0)],
                5: [qk(1, 1, 1, 1)], 6: [qk(0, 1, 1, 0)],
                7: [qk(0, 1, 1, 1)],
            })
            stB = drain_copies(pyB)

            # ---- block C: pair1/i0 (PE-light)
            pyC = attention_block(1, 0, sched={
                0: [lambda: drain_norm(stB, 0, 1)],
            })
            stC = drain_copies(pyC)

            # ---- block D: pair1/i1 with i0 out-projection interleaved
            ot0 = opool.tile([P, 4, 512], FP16, tag="ot")
            _d_po = {}

            def op0(m, phase):
                def fn():
                    if phase == 0:
                        _d_po[m] = po_tile()
                        outproj_mm(m, 0, _d_po[m], slice(0, 512))
                    elif phase == 1:
                        outproj_mm(m, 1, _d_po[m], slice(0, 512))
                    else:
                        nc.vector.tensor_copy(out=ot0[:, m, :],
                                              in_=_d_po[m])
                return fn

            pyD = attention_block(1, 1, sched={
                0: [lambda: drain_norm(stC, 1, 0)],
                1: [op0(0, 0)], 2: [op0(0, 1), op0(0, 2)],
                3: [op0(1, 0), op0(1, 1)], 4: [op0(1, 2), op0(2, 0)],
                5: [op0(2, 1), op0(2, 2)], 6: [op0(3, 0), op0(3, 1)],
                7: [op0(3, 2)],
            })
            out_t = out.rearrange("(m p) n -> p m n", p=P)
            nc.sync.dma_start(out=out_t[:, :, 0:512], in_=ot0)

            # ---- tail: i1 out-projection; g=0 matmuls fill the PE while
            # the last drain chain runs, then g=1 + copies + store.
            tail_po = [po_tile() if m < 2 else po_tile(tag="s")
                       for m in range(4)]
            for m in range(4):
                outproj_mm(m, 0, tail_po[m], slice(512, 1024))
            stD = drain_copies(pyD)
            drain_norm(stD, 1, 1, pr_tag="py0")
            ot1 = opool.tile([P, 4, 512], FP16, tag="ot")
            for m in range(4):
                outproj_mm(m, 1, tail_po[m], slice(512, 1024))
                nc.vector.tensor_copy(out=ot1[:, m, :], in_=tail_po[m])
            nc.sync.dma_start(out=out_t[:, :, 512:1024], in_=ot1)

    nc.compile()
    return nc


def get_nc():
    if "nc" not in _NC_CACHE:
        _NC_CACHE["nc"] = build_nc()
    return _NC_CACHE["nc"]


def make_in_maps(x, Wq, Wk, Wv, Wo):
    in_maps = []
    for core in range(8):
        b, g = core // 2, core % 2
        sl = slice(g * 256, (g + 1) * 256)
        wqkv = np.stack(
            [Wq[sl, :].T, Wk[sl, :].T, Wv[sl, :].T], axis=1
        )  # (512, 3, 256)
        # paired Wo layout: wo_p[d + 64*(h%2), h//2, c_out]
        wo_l = Wo[:, sl].reshape(C, HL, D)          # (c_out, h, d)
        wo_p = np.empty((P, 2, C), dtype=np.float32)
        for h in range(HL):
            wo_p[(h % 2) * D:(h % 2) * D + D, h // 2, :] = wo_l[:, h, :].T
        in_maps.append({
            "x": np.ascontiguousarray(x[b]).astype(np.float16),
            "wqkv_t": np.ascontiguousarray(wqkv).astype(np.float16),
            "wo_p": np.ascontiguousarray(wo_p).astype(np.float16),
        })
    return in_maps


LAST_RESULTS = {}


def kernel(x, Wq, Wk, Wv, Wo, _trace=False):
    x = np.asarray(x, dtype=np.float32)
    Wq = np.asarray(Wq, dtype=np.float32)
    Wk = np.asarray(Wk, dtype=np.float32)
    Wv = np.asarray(Wv, dtype=np.float32)
    Wo = np.asarray(Wo, dtype=np.float32)

    nc = get_nc()
    in_maps = make_in_maps(x, Wq, Wk, Wv, Wo)
    res = run_bass_kernel_spmd(
        nc, in_maps, core_ids=list(range(8)), trace=_trace
    )
    LAST_RESULTS["res"] = res
    parts = [np.asarray(r["out_p"]).astype(np.float32) for r in res.results]
    out = np.stack([parts[2 * b] + parts[2 * b + 1] for b in range(4)])
    return out


# revision 14
# speedup vs baseline: 1.2668x; 1.0650x over previous
"""Multi-head attention Trainium2 Bass kernel (v3, fp16/f32r).

Problem: x:(4,512,1024), Wq/Wk/Wv/Wo:(512,512), H=8 heads, d=64.
  q = Wq@x ; k = Wk@x ; v = Wv@x  (1x1 conv == channel matmul)
  per head: S[i,j] = q[:,i].k[:,j] ; attn = softmax_j(S) ; y = attn @ v
  out = Wo @ y

Sharding: 8 cores = (batch b, head-half g).  Core (b,g) handles batch b,
local heads g*4..g*4+3 and computes the partial output
out_p = Wo[:, g*256:(g+1)*256] @ y_g which the host sums pairwise.

Design notes (v3):
- Inputs/projections in fp16 (host-cast): half the DMA of f32, 1
  row/cycle on the PE, and an 11-bit mantissa so score differences stay
  accurate (bf16 cost 1.3e-2 rel err; fp16 ~1e-3).  exp output e and v
  stay f32r (range to e^52 and ACT writes f32 faster than bf16).
- Scores are computed transposed (S^T = k^T q) per head pair with the
  two head lanes ROW-TILED onto PE row groups 0-63 / 64-127 (auto
  tile_position) so both K=64 matmuls run concurrently.
- The ACT-engine exp stream (32 x [128,1024] ~ 36us) is the floor; the
  emission order keeps it dense: block A (pair0/i0) interleaves the
  just-in-time V projection and the pair0 nn=1 QK groups, block B
  (pair0/i1) the pair1 QK projection, block D (pair1/i1) the i0 output
  projection.  Each block's softmax-denominator/normalize chain is
  deferred into the next block so it never stalls the PE FIFO.
- PV keeps the ones-column trick (M=65): row 64 of the PV accumulator
  is the denominator.  It is broadcast across partitions by two K=1
  col-tiled matmuls (rhs = the denominator rows in SBUF), reciprocal on
  DVE, normalize on GPSIMD into a pair-stacked y layout [128, n]
  (head 2g on partitions 0-63, head 2g+1 on 64-127, via a partition-
  shifting PSUM->SBUF copy).
- With y pair-stacked, the output projection is a plain K=128 matmul
  per (m, g): the contraction sums the head pair directly.
"""

import numpy as np

import concourse.bass as bass
import concourse.tile as tile
from concourse import bacc
from concourse import mybir
from concourse.bass_utils import run_bass_kernel_spmd

F32 = mybir.dt.float32
F32R = mybir.dt.float32r
FP16 = mybir.dt.float16

P = 128
C = 512          # channels
NSEQ = 1024      # sequence length
D = 64           # head dim
HL = 4           # local heads per core
KC = C // P      # 4 contraction tiles over channels
J = NSEQ // P    # 8 key tiles

_NC_CACHE = {}


def build_nc():
    nc = bacc.Bacc("TRN2")

    x = nc.dram_tensor("x", [C, NSEQ], FP16, kind="ExternalInput")
    wqkv = nc.dram_tensor("wqkv_t", [C, 3, 2 * P], FP16, kind="ExternalInput")
    # paired Wo: [d + 64*(h%2), h//2, c_out]
    wo = nc.dram_tensor("wo_p", [P, 2, C], FP16, kind="ExternalInput")
    out = nc.dram_tensor("out_p", [C, NSEQ], FP16, kind="ExternalOutput")

    with tile.TileContext(nc) as tc:
        with (
            tc.tile_pool(name="consts", bufs=1) as consts,
            tc.tile_pool(name="epool", bufs=4) as epool,
            tc.tile_pool(name="ypool", bufs=3) as ypool,
            tc.tile_pool(name="rpool", bufs=2) as rpool,
            tc.tile_pool(name="opool", bufs=2) as opool,
            tc.tile_pool(name="pp", bufs=2, space="PSUM") as pp,
        ):
            # ---- persistent tiles
            warm = consts.tile([P, 512], FP16)
            # selector for the denominator broadcast: a K=33 matmul with
            # lhsT rows 0/32 routes lane0's denom to partitions 0-63 and
            # lane1's to 64-127 in one (0,0)-positioned matmul.
            sel = consts.tile([33, P], F32R)
            dd = consts.tile([33, 512], F32R)
            w_sb = [consts.tile([P, 3, 2 * P], FP16, name=f"w{kc}")
                    for kc in range(KC)]
            x_sb = [[consts.tile([P, 512], FP16, name=f"x{kc}_{sh}")
                     for sh in range(2)] for kc in range(KC)]
            wot_sb = consts.tile([P, 2, C], FP16)
            q_sb = [[consts.tile([P, 512], FP16, name=f"q{m}_{sh}")
                     for sh in range(2)] for m in range(2)]
            k_sb = [[consts.tile([P, 512], FP16, name=f"k{m}_{sh}")
                     for sh in range(2)] for m in range(2)]
            vt_sb = [consts.tile([P, HL, D + 1], F32R, name=f"vt{j}")
                     for j in range(J)]
            # pair-stacked y: yt[g][64*lane + d, n] = y_{head 2g+lane}[d, n]
            yt = [consts.tile([P, NSEQ], FP16, name=f"yt{g}")
                  for g in range(2)]

            # ---- init constants (gpsimd; keeps DVE/ACT free)
            nc.gpsimd.memset(warm, 0)
            # memset can't target f32r; write the f32 bit pattern of 1.0
            # through a uint32 view instead.
            ONE_F32 = 0x3F800000
            nc.gpsimd.memset(sel.bitcast(mybir.dt.uint32), 0)
            nc.gpsimd.memset(sel[0:1, 0:D].bitcast(mybir.dt.uint32), ONE_F32)
            nc.gpsimd.memset(sel[32:33, D:2 * D].bitcast(mybir.dt.uint32),
                             ONE_F32)
            # rows 1-31 of dd are never written; zero them once so the
            # 0-weight selector rows can't hit NaNs in the K=33 matmul.
            nc.gpsimd.memset(dd.bitcast(mybir.dt.uint32), 0)
            for j in range(J):
                nc.gpsimd.memset(
                    vt_sb[j][:, :, D:D + 1].bitcast(mybir.dt.uint32), ONE_F32)

            def po_tile(name="po", tag="po"):
                return pp.tile([P, 512], F32, tag=tag, name=name, bufs=2)

            # ---- warm-up matmuls release the HAM clock gate during load
            for wi in range(8):
                nc.tensor.matmul(po_tile("pw"), lhsT=warm[:, 0:P], rhs=warm,
                                 start=True, stop=True)

            # ---- input DMAs, alternating dispatch engines (SP/ACT) so
            # descriptor writes don't serialize; order = first-needed.
            w_t = wqkv.rearrange("(kc p) w m -> p kc w m", p=P)
            x_t = x.rearrange("(kc p) (sh n) -> p kc sh n", p=P, n=512)
            loads = []
            for kc in range(KC):
                loads.append((w_sb[kc], w_t[:, kc]))
            for sh in range(2):
                for kc in range(KC):
                    loads.append((x_sb[kc][sh], x_t[:, kc, sh]))
            loads.append((wot_sb, wo[:, :, :]))
            for i, (dst, src) in enumerate(loads):
                eng = nc.sync if i % 2 == 0 else nc.scalar
                eng.dma_start(dst, src)

            # ---- QK projections: group = (w_idx, m, nn), 4 matmuls
            def proj_group(w_idx, m, nn, split=None):
                """split=None: all 4 kc matmuls + copy. split=0: first 2.
                split=1: last 2 + copy."""
                key = (w_idx, m, nn)
                if split in (None, 0):
                    _proj_ps[key] = po_tile()
                ps = _proj_ps[key]
                kcs = range(KC) if split is None else (
                    (0, 1) if split == 0 else (2, 3))
                for kc in kcs:
                    nc.tensor.matmul(
                        ps,
                        lhsT=w_sb[kc][:, w_idx, m * P:(m + 1) * P],
                        rhs=x_sb[kc][nn],
                        start=(kc == 0),
                        stop=(kc == KC - 1),
                    )
                if split in (None, 1):
                    dst = (q_sb if w_idx == 0 else k_sb)[m][nn]
                    nc.vector.tensor_copy(out=dst, in_=ps)

            _proj_ps = {}
            proj_group(0, 0, 0)     # q pair0, seq-half 0
            proj_group(1, 0, 0)     # k pair0, seq-half 0

            def vproj(j):
                psv = po_tile()
                for kc in range(KC):
                    nc.tensor.matmul(
                        psv[:, 0:2 * P],
                        lhsT=x_sb[kc][j // 4][:, (j % 4) * P:(j % 4 + 1) * P],
                        rhs=w_sb[kc][:, 2, :],
                        start=(kc == 0),
                        stop=(kc == KC - 1),
                    )
                nc.vector.tensor_copy(
                    out=vt_sb[j][:, :, 0:D],
                    in_=psv[:, 0:2 * P].rearrange("p (h d) -> p h d", h=HL),
                )

            def scores_exp(pair, ihalf, j):
                ps = pp.tile([P, 1024], F32, tag="s", name="s", bufs=2)
                for lane in range(2):
                    hp = lane * D
                    nc.tensor.matmul(
                        ps[:, lane * 512:(lane + 1) * 512],
                        lhsT=k_sb[pair][j // 4][hp:hp + D,
                                                (j % 4) * P:(j % 4 + 1) * P],
                        rhs=q_sb[pair][ihalf][hp:hp + D, :],
                        start=True, stop=True,
                    )
                e = epool.tile([P, 1024], F32R, tag="e", name="e")
                nc.scalar.activation(
                    out=e, in_=ps,
                    func=mybir.ActivationFunctionType.Exp,
                )
                return e

            def make_pv(py, pair, j, e):
                def fn():
                    for lane in range(2):
                        nc.tensor.matmul(
                            py[lane][0:D + 1, :],
                            lhsT=vt_sb[j][:, 2 * pair + lane, :],
                            rhs=e[:, lane * 512:(lane + 1) * 512],
                            start=(j == 0), stop=(j == J - 1),
                        )
                return fn

            def drain_copies(py):
                # lane0: y rows at base 0
                yu0 = ypool.tile([D, 512], F32R, tag="yu", name="yu")
                nc.vector.tensor_copy(out=yu0, in_=py[0][0:D, :])
                # lane1: y rows shifted to the upper partition half so the
                # normalize + stacked out-projection stay base-aligned
                yu1 = ypool.tile([P, 512], F32R, tag="yu1", name="yu1")
                nc.vector.tensor_copy(out=yu1[D:2 * D, :],
                                      in_=py[1][0:D, :])
                # denominator rows into the shared selector rhs tile
                nc.vector.tensor_copy(out=dd[0:1, :], in_=py[0][D:D + 1, :])
                nc.vector.tensor_copy(out=dd[32:33, :],
                                      in_=py[1][D:D + 1, :])
                return (yu0, yu1)

            def drain_norm(bi, pair, ihalf, pr_tag="po"):
                yu0, yu1 = st[bi]
                isl = slice(ihalf * 512, (ihalf + 1) * 512)
                # denominator broadcast: one K=33 selector matmul
                pr = pp.tile([P, 512], F32, tag=pr_tag, name="pr",
                             bufs=2 if pr_tag == "po" else 1)
                nc.tensor.matmul(
                    pr, lhsT=sel, rhs=dd, start=True, stop=True,
                )
                rr = rpool.tile([P, 512], F32, tag="rr", name="rr")
                nc.vector.reciprocal_approx_fast(out=rr, in_=pr)
                # normalize into the pair-stacked y tile; the two lanes go
                # to different engines so they run concurrently
                nc.gpsimd.tensor_tensor(
                    out=yt[pair][0:D, isl],
                    in0=yu0, in1=rr[0:D, :],
                    op=mybir.AluOpType.mult,
                )
                nc.vector.tensor_tensor(
                    out=yt[pair][D:2 * D, isl],
                    in0=yu1[D:2 * D, :], in1=rr[D:2 * D, :],
                    op=mybir.AluOpType.mult,
                )

            def outproj_mm(m, g, po, isl):
                # stacked K=128 contraction sums the head pair directly
                nc.tensor.matmul(
                    po,
                    lhsT=wot_sb[:, g, m * P:(m + 1) * P],
                    rhs=yt[g][:, isl],
                    start=(g == 0), stop=(g == 1),
                )

            def qk(w_idx, m, nn, split):
                return lambda: proj_group(w_idx, m, nn, split=split)

            def vp(j):
                return lambda: vproj(j)

            ot0 = opool.tile([P, 4, 512], FP16, tag="ot")
            _d_po = {}

            def op0(m, phase):
                def fn():
                    if phase == 0:
                        _d_po[m] = po_tile()
                        outproj_mm(m, 0, _d_po[m], slice(0, 512))
                    elif phase == 1:
                        outproj_mm(m, 1, _d_po[m], slice(0, 512))
                    else:
                        nc.vector.tensor_copy(out=ot0[:, m, :],
                                              in_=_d_po[m])
                return fn

            st = {}
            norm = drain_norm
            blocks = [
                # (pair, ihalf, sched) — sched[j] runs between exp_j and
                # the (lagged) PV of the previous iteration
                (0, 0, {
                    0: [vp(0), vp(1)], 1: [vp(2), qk(1, 0, 1, 0)],
                    2: [vp(3), qk(1, 0, 1, 1)], 3: [vp(4), qk(0, 0, 1, 0)],
                    4: [vp(5), qk(0, 0, 1, 1)], 5: [vp(6)], 6: [vp(7)],
                }),
                (0, 1, {
                    0: [qk(1, 1, 0, 0)], 1: [qk(1, 1, 0, 1)],
                    2: [lambda: norm(0, 0, 0), qk(0, 1, 0, 0)],
                    3: [qk(0, 1, 0, 1)], 4: [qk(1, 1, 1, 0)],
                    5: [qk(1, 1, 1, 1)], 6: [qk(0, 1, 1, 0)],
                    7: [qk(0, 1, 1, 1)],
                }),
                (1, 0, {
                    2: [lambda: norm(1, 0, 1)],
                }),
                (1, 1, {
                    1: [lambda: norm(2, 1, 0)],
                    3: [op0(0, 0)], 4: [op0(0, 1), op0(0, 2)],
                    5: [op0(1, 0), op0(1, 1)],
                    6: [op0(1, 2), op0(2, 0), op0(2, 1)],
                    7: [op0(2, 2), op0(3, 0), op0(3, 1)],
                }),
            ]

            # ---- the flattened 32-iteration pipeline: PV is emitted one
            # iteration late so the in-order PE queue never waits on exp.
            pys = {}
            pending = None
            for bi, (pair, ihalf, sched) in enumerate(blocks):
                pys[bi] = [
                    pp.tile([P, 512], F32, tag="py0", name="py0", bufs=1),
                    pp.tile([P, 512], F32, tag="py1", name="py1", bufs=1),
                ]
                for j in range(J):
                    e = scores_exp(pair, ihalf, j)
                    for fn in sched.get(j, ()):
                        fn()
                    if pending is not None:
                        pending()
                    if j == 0 and bi > 0:
                        st[bi - 1] = drain_copies(pys[bi - 1])
                    pending = make_pv(pys[bi], pair, j, e)

            # ---- tail: i1 out-projection g=0 fills the PE while the last
            # PV + drain chain run; then g=1, copies, stores.
            out_t = out.rearrange("(m p) n -> p m n", p=P)
            tail_po = [po_tile() if m < 2 else po_tile(tag="s")
                       for m in range(4)]
            for m in range(4):
                outproj_mm(m, 0, tail_po[m], slice(512, 1024))
            pending()
            op0(3, 2)()
            nc.sync.dma_start(out=out_t[:, :, 0:512], in_=ot0)
            st[3] = drain_copies(pys[3])
            drain_norm(3, 1, 1, pr_tag="py0")
            ot1 = opool.tile([P, 4, 512], FP16, tag="ot")
            for m in range(4):
                outproj_mm(m, 1, tail_po[m], slice(512, 1024))
                nc.vector.tensor_copy(out=ot1[:, m, :], in_=tail_po[m])
                nc.sync.dma_start(out=out_t[:, m, 512:1024],
                                  in_=ot1[:, m, :])

    nc.compile()
    return nc


def get_nc():
    if "nc" not in _NC_CACHE:
        _NC_CACHE["nc"] = build_nc()
    return _NC_CACHE["nc"]


def make_in_maps(x, Wq, Wk, Wv, Wo):
    in_maps = []
    for core in range(8):
        b, g = core // 2, core % 2
        sl = slice(g * 256, (g + 1) * 256)
        wqkv = np.stack(
            [Wq[sl, :].T, Wk[sl, :].T, Wv[sl, :].T], axis=1
        )  # (512, 3, 256)
        # paired Wo layout: wo_p[d + 64*(h%2), h//2, c_out]
        wo_l = Wo[:, sl].reshape(C, HL, D)          # (c_out, h, d)
        wo_p = np.empty((P, 2, C), dtype=np.float32)
        for h in range(HL):
            wo_p[(h % 2) * D:(h % 2) * D + D, h // 2, :] = wo_l[:, h, :].T
        in_maps.append({
            "x": np.ascontiguousarray(x[b]).astype(np.float16),
            "wqkv_t": np.ascontiguousarray(wqkv).astype(np.float16),
            "wo_p": np.ascontiguousarray(wo_p).astype(np.float16),
        })
    return in_maps


LAST_RESULTS = {}


def kernel(x, Wq, Wk, Wv, Wo, _trace=False):
    x = np.asarray(x, dtype=np.float32)
    Wq = np.asarray(Wq, dtype=np.float32)
    Wk = np.asarray(Wk, dtype=np.float32)
    Wv = np.asarray(Wv, dtype=np.float32)
    Wo = np.asarray(Wo, dtype=np.float32)

    nc = get_nc()
    in_maps = make_in_maps(x, Wq, Wk, Wv, Wo)
    res = run_bass_kernel_spmd(
        nc, in_maps, core_ids=list(range(8)), trace=_trace
    )
    LAST_RESULTS["res"] = res
    parts = [np.asarray(r["out_p"]).astype(np.float32) for r in res.results]
    out = np.stack([parts[2 * b] + parts[2 * b + 1] for b in range(4)])
    return out


# revision 16
# speedup vs baseline: 1.3101x; 1.0342x over previous
"""Multi-head attention Trainium2 Bass kernel (v3, fp16/f32r).

Problem: x:(4,512,1024), Wq/Wk/Wv/Wo:(512,512), H=8 heads, d=64.
  q = Wq@x ; k = Wk@x ; v = Wv@x  (1x1 conv == channel matmul)
  per head: S[i,j] = q[:,i].k[:,j] ; attn = softmax_j(S) ; y = attn @ v
  out = Wo @ y

Sharding: 8 cores = (batch b, head-half g).  Core (b,g) handles batch b,
local heads g*4..g*4+3 and computes the partial output
out_p = Wo[:, g*256:(g+1)*256] @ y_g which the host sums pairwise.

Design notes (v3):
- Inputs/projections in fp16 (host-cast): half the DMA of f32, 1
  row/cycle on the PE, and an 11-bit mantissa so score differences stay
  accurate (bf16 cost 1.3e-2 rel err; fp16 ~1e-3).  exp output e and v
  stay f32r (range to e^52 and ACT writes f32 faster than bf16).
- Scores are computed transposed (S^T = k^T q) per head pair with the
  two head lanes ROW-TILED onto PE row groups 0-63 / 64-127 (auto
  tile_position) so both K=64 matmuls run concurrently.
- The ACT-engine exp stream (32 x [128,1024] ~ 36us) is the floor; the
  emission order keeps it dense: block A (pair0/i0) interleaves the
  just-in-time V projection and the pair0 nn=1 QK groups, block B
  (pair0/i1) the pair1 QK projection, block D (pair1/i1) the i0 output
  projection.  Each block's softmax-denominator/normalize chain is
  deferred into the next block so it never stalls the PE FIFO.
- PV keeps the ones-column trick (M=65): row 64 of the PV accumulator
  is the denominator.  It is broadcast across partitions by two K=1
  col-tiled matmuls (rhs = the denominator rows in SBUF), reciprocal on
  DVE, normalize on GPSIMD into a pair-stacked y layout [128, n]
  (head 2g on partitions 0-63, head 2g+1 on 64-127, via a partition-
  shifting PSUM->SBUF copy).
- With y pair-stacked, the output projection is a plain K=128 matmul
  per (m, g): the contraction sums the head pair directly.
"""

import numpy as np

import concourse.bass as bass
import concourse.tile as tile
from concourse import bacc
from concourse import mybir
from concourse.bass_utils import run_bass_kernel_spmd

F32 = mybir.dt.float32
F32R = mybir.dt.float32r
FP16 = mybir.dt.float16

P = 128
C = 512          # channels
NSEQ = 1024      # sequence length
D = 64           # head dim
HL = 4           # local heads per core
KC = C // P      # 4 contraction tiles over channels
J = NSEQ // P    # 8 key tiles

_NC_CACHE = {}


def build_nc():
    nc = bacc.Bacc("TRN2")

    x = nc.dram_tensor("x", [C, NSEQ], FP16, kind="ExternalInput")
    wqkv = nc.dram_tensor("wqkv_t", [C, 3, 2 * P], FP16, kind="ExternalInput")
    # paired Wo: [d + 64*(h%2), h//2, c_out]
    wo = nc.dram_tensor("wo_p", [P, 2, C], FP16, kind="ExternalInput")
    out = nc.dram_tensor("out_p", [C, NSEQ], FP16, kind="ExternalOutput")

    with tile.TileContext(nc) as tc:
        with (
            tc.tile_pool(name="consts", bufs=1) as consts,
            tc.tile_pool(name="epool", bufs=4) as epool,
            tc.tile_pool(name="ypool", bufs=3) as ypool,
            tc.tile_pool(name="rpool", bufs=2) as rpool,
            tc.tile_pool(name="opool", bufs=2) as opool,
            tc.tile_pool(name="pp", bufs=2, space="PSUM") as pp,
        ):
            # ---- persistent tiles
            warm = consts.tile([P, 512], FP16)
            # selector for the denominator broadcast: a K=33 matmul with
            # lhsT rows 0/32 routes lane0's denom to partitions 0-63 and
            # lane1's to 64-127 in one (0,0)-positioned matmul.
            sel = consts.tile([33, P], F32R)
            dd = consts.tile([33, 512], F32R)
            w_sb = [consts.tile([P, 3, 2 * P], FP16, name=f"w{kc}")
                    for kc in range(KC)]
            x_sb = [[consts.tile([P, 512], FP16, name=f"x{kc}_{sh}")
                     for sh in range(2)] for kc in range(KC)]
            wot_sb = consts.tile([P, 2, C], FP16)
            q_sb = [[consts.tile([P, 512], FP16, name=f"q{m}_{sh}")
                     for sh in range(2)] for m in range(2)]
            k_sb = [[consts.tile([P, 512], FP16, name=f"k{m}_{sh}")
                     for sh in range(2)] for m in range(2)]
            vt_sb = [consts.tile([P, HL, D + 1], F32R, name=f"vt{j}")
                     for j in range(J)]
            # pair-stacked y: yt[g][64*lane + d, n] = y_{head 2g+lane}[d, n]
            yt = [consts.tile([P, NSEQ], FP16, name=f"yt{g}")
                  for g in range(2)]

            # ---- init constants (gpsimd; keeps DVE/ACT free)
            nc.gpsimd.memset(warm, 0)
            # memset can't target f32r; write the f32 bit pattern of 1.0
            # through a uint32 view instead.
            ONE_F32 = 0x3F800000
            nc.gpsimd.memset(sel.bitcast(mybir.dt.uint32), 0)
            nc.gpsimd.memset(sel[0:1, 0:D].bitcast(mybir.dt.uint32), ONE_F32)
            nc.gpsimd.memset(sel[32:33, D:2 * D].bitcast(mybir.dt.uint32),
                             ONE_F32)
            # rows 1-31 of dd are never written; zero them once so the
            # 0-weight selector rows can't hit NaNs in the K=33 matmul.
            nc.gpsimd.memset(dd.bitcast(mybir.dt.uint32), 0)
            for j in range(J):
                nc.gpsimd.memset(
                    vt_sb[j][:, :, D:D + 1].bitcast(mybir.dt.uint32), ONE_F32)

            def po_tile(name="po", tag="po"):
                return pp.tile([P, 512], F32, tag=tag, name=name, bufs=2)

            # ---- warm-up matmuls release the HAM clock gate during load
            for wi in range(12):
                nc.tensor.matmul(po_tile("pw"), lhsT=warm[:, 0:P], rhs=warm,
                                 start=True, stop=True)

            # ---- input DMAs, alternating dispatch engines (SP/ACT) so
            # descriptor writes don't serialize; order = first-needed.
            w_t = wqkv.rearrange("(kc p) w m -> p kc w m", p=P)
            x_t = x.rearrange("(kc p) (sh n) -> p kc sh n", p=P, n=512)
            loads = []
            for kc in range(KC):
                loads.append((w_sb[kc], w_t[:, kc]))
            for kc in range(KC):
                loads.append((x_sb[kc][0], x_t[:, kc, 0]))

            def emit_loads(lds):
                for i, (dst, src) in enumerate(lds):
                    eng = nc.sync if i % 2 == 0 else nc.scalar
                    eng.dma_start(dst, src)

            emit_loads(loads)
            loads2 = [(x_sb[kc][1], x_t[:, kc, 1]) for kc in range(KC)]
            loads2.append((wot_sb, wo[:, :, :]))

            # ---- QK projections: group = (w_idx, m, nn), 4 matmuls
            def proj_group(w_idx, m, nn, split=None):
                """split=None: all 4 kc matmuls + copy. split=0: first 2.
                split=1: last 2 + copy."""
                key = (w_idx, m, nn)
                if split in (None, 0):
                    _proj_ps[key] = po_tile()
                ps = _proj_ps[key]
                kcs = range(KC) if split is None else (
                    (0, 1) if split == 0 else (2, 3))
                for kc in kcs:
                    nc.tensor.matmul(
                        ps,
                        lhsT=w_sb[kc][:, w_idx, m * P:(m + 1) * P],
                        rhs=x_sb[kc][nn],
                        start=(kc == 0),
                        stop=(kc == KC - 1),
                    )
                if split in (None, 1):
                    dst = (q_sb if w_idx == 0 else k_sb)[m][nn]
                    nc.vector.tensor_copy(out=dst, in_=ps)

            _proj_ps = {}
            proj_group(0, 0, 0)     # q pair0, seq-half 0
            proj_group(1, 0, 0)     # k pair0, seq-half 0
            emit_loads(loads2)      # seq-half 1 + wo after the first projs

            def vproj(j):
                psv = po_tile()
                for kc in range(KC):
                    nc.tensor.matmul(
                        psv[:, 0:2 * P],
                        lhsT=x_sb[kc][j // 4][:, (j % 4) * P:(j % 4 + 1) * P],
                        rhs=w_sb[kc][:, 2, :],
                        start=(kc == 0),
                        stop=(kc == KC - 1),
                    )
                nc.vector.tensor_copy(
                    out=vt_sb[j][:, :, 0:D],
                    in_=psv[:, 0:2 * P].rearrange("p (h d) -> p h d", h=HL),
                )

            def scores_exp(pair, ihalf, j):
                ps = pp.tile([P, 1024], F32, tag="s", name="s", bufs=2)
                for lane in range(2):
                    hp = lane * D
                    nc.tensor.matmul(
                        ps[:, lane * 512:(lane + 1) * 512],
                        lhsT=k_sb[pair][j // 4][hp:hp + D,
                                                (j % 4) * P:(j % 4 + 1) * P],
                        rhs=q_sb[pair][ihalf][hp:hp + D, :],
                        start=True, stop=True,
                    )
                e = epool.tile([P, 1024], F32R, tag="e", name="e")
                nc.scalar.activation(
                    out=e, in_=ps,
                    func=mybir.ActivationFunctionType.Exp,
                )
                return e

            def make_pv(py, pair, j, e):
                def fn():
                    for lane in range(2):
                        nc.tensor.matmul(
                            py[lane][0:D + 1, :],
                            lhsT=vt_sb[j][:, 2 * pair + lane, :],
                            rhs=e[:, lane * 512:(lane + 1) * 512],
                            start=(j == 0), stop=(j == J - 1),
                        )
                return fn

            def drain_copies(py, dd_on_act=False):
                # lane0: y rows at base 0
                yu0 = ypool.tile([D, 512], F32R, tag="yu", name="yu")
                nc.vector.tensor_copy(out=yu0, in_=py[0][0:D, :])
                # lane1: y rows shifted to the upper partition half so the
                # normalize + stacked out-projection stay base-aligned
                yu1 = ypool.tile([P, 512], F32R, tag="yu1", name="yu1")
                nc.vector.tensor_copy(out=yu1[D:2 * D, :],
                                      in_=py[1][0:D, :])
                # denominator rows into the shared selector rhs tile
                if dd_on_act:
                    nc.scalar.activation(
                        out=dd[0:1, :], in_=py[0][D:D + 1, :],
                        func=mybir.ActivationFunctionType.Identity)
                    nc.scalar.activation(
                        out=dd[32:33, :], in_=py[1][D:D + 1, :],
                        func=mybir.ActivationFunctionType.Identity)
                else:
                    nc.vector.tensor_copy(out=dd[0:1, :],
                                          in_=py[0][D:D + 1, :])
                    nc.vector.tensor_copy(out=dd[32:33, :],
                                          in_=py[1][D:D + 1, :])
                return (yu0, yu1)

            def drain_norm(bi, pair, ihalf, pr_tag="po", all_dve=False):
                yu0, yu1 = st[bi]
                isl = slice(ihalf * 512, (ihalf + 1) * 512)
                # denominator broadcast: one K=33 selector matmul
                pr = pp.tile([P, 512], F32, tag=pr_tag, name="pr",
                             bufs=2 if pr_tag == "po" else 1)
                nc.tensor.matmul(
                    pr, lhsT=sel, rhs=dd, start=True, stop=True,
                )
                rr = rpool.tile([P, 512], F32, tag="rr", name="rr")
                nc.vector.reciprocal_approx_fast(out=rr, in_=pr)
                # normalize into the pair-stacked y tile; the two lanes go
                # to different engines so they run concurrently
                eng0 = nc.vector if all_dve else nc.gpsimd
                eng0.tensor_tensor(
                    out=yt[pair][0:D, isl],
                    in0=yu0, in1=rr[0:D, :],
                    op=mybir.AluOpType.mult,
                )
                nc.vector.tensor_tensor(
                    out=yt[pair][D:2 * D, isl],
                    in0=yu1[D:2 * D, :], in1=rr[D:2 * D, :],
                    op=mybir.AluOpType.mult,
                )

            def outproj_mm(m, g, po, isl):
                # stacked K=128 contraction sums the head pair directly
                nc.tensor.matmul(
                    po,
                    lhsT=wot_sb[:, g, m * P:(m + 1) * P],
                    rhs=yt[g][:, isl],
                    start=(g == 0), stop=(g == 1),
                )

            def qk(w_idx, m, nn, split):
                return lambda: proj_group(w_idx, m, nn, split=split)

            def vp(j):
                return lambda: vproj(j)

            ot0 = opool.tile([P, 4, 512], FP16, tag="ot")
            _d_po = {}

            def op0(m, phase):
                def fn():
                    if phase == 0:
                        _d_po[m] = po_tile()
                        outproj_mm(m, 0, _d_po[m], slice(0, 512))
                    elif phase == 1:
                        outproj_mm(m, 1, _d_po[m], slice(0, 512))
                    else:
                        nc.vector.tensor_copy(out=ot0[:, m, :],
                                              in_=_d_po[m])
                return fn

            st = {}
            norm = drain_norm
            blocks = [
                # (pair, ihalf, sched) — sched[j] runs between exp_j and
                # the (lagged) PV of the previous iteration.  Each block
                # interleaves only the projections it (or the next block)
                # needs soonest, balancing PE load across the exp stream.
                (0, 0, {
                    0: [vp(0), vp(1)], 1: [vp(2), qk(1, 0, 1, 0)],
                    2: [vp(3), qk(1, 0, 1, 1)], 3: [vp(4), qk(0, 0, 1, 0)],
                    4: [vp(5), qk(0, 0, 1, 1)], 5: [vp(6)], 6: [vp(7)],
                }),
                (0, 1, {
                    0: [qk(1, 1, 0, 0)], 1: [qk(1, 1, 0, 1)],
                    2: [lambda: norm(0, 0, 0), qk(0, 1, 0, 0)],
                    3: [qk(0, 1, 0, 1)],
                }),
                (1, 0, {
                    0: [qk(1, 1, 1, 0)], 1: [qk(1, 1, 1, 1)],
                    2: [lambda: norm(1, 0, 1), qk(0, 1, 1, 0)],
                    3: [qk(0, 1, 1, 1)],
                }),
                (1, 1, {
                    1: [lambda: norm(2, 1, 0)],
                    3: [op0(0, 0)], 4: [op0(0, 1), op0(0, 2)],
                    5: [op0(1, 0), op0(1, 1)],
                    6: [op0(1, 2), op0(2, 0), op0(2, 1)],
                    7: [op0(2, 2), op0(3, 0), op0(3, 1)],
                }),
            ]

            # ---- the flattened 32-iteration pipeline: PV is emitted one
            # iteration late so the in-order PE queue never waits on exp.
            pys = {}
            pending = None
            for bi, (pair, ihalf, sched) in enumerate(blocks):
                pys[bi] = [
                    pp.tile([P, 512], F32, tag="py0", name="py0", bufs=1),
                    pp.tile([P, 512], F32, tag="py1", name="py1", bufs=1),
                ]
                for j in range(J):
                    e = scores_exp(pair, ihalf, j)
                    for fn in sched.get(j, ()):
                        fn()
                    if pending is not None:
                        pending()
                    if j == 0 and bi > 0:
                        st[bi - 1] = drain_copies(pys[bi - 1])
                    pending = make_pv(pys[bi], pair, j, e)

            # ---- tail: i1 out-projection g=0 fills the PE while the last
            # PV + drain chain run; then g=1, copies, stores.
            out_t = out.rearrange("(m p) n -> p m n", p=P)
            tail_po = [po_tile() if m < 2 else po_tile(tag="s")
                       for m in range(4)]
            for m in range(4):
                outproj_mm(m, 0, tail_po[m], slice(512, 1024))
            pending()
            st[3] = drain_copies(pys[3], dd_on_act=True)
            # PE filler keeps HAM at 8/8 through the drain window so the
            # final projections run at full clock
            for wi in range(5):
                nc.tensor.matmul(pp.tile([P, 512], F32, tag="py1",
                                         name="pf", bufs=1),
                                 lhsT=warm[:, 0:P], rhs=warm,
                                 start=True, stop=True)
            drain_norm(3, 1, 1, pr_tag="py0", all_dve=True)
            op0(3, 2)()
            nc.sync.dma_start(out=out_t[:, :, 0:512], in_=ot0)
            ot1 = opool.tile([P, 4, 512], FP16, tag="ot")
            for m in range(4):
                outproj_mm(m, 1, tail_po[m], slice(512, 1024))
                nc.vector.tensor_copy(out=ot1[:, m, :], in_=tail_po[m])
                nc.sync.dma_start(out=out_t[:, m, 512:1024],
                                  in_=ot1[:, m, :])

    nc.compile()
    return nc


def get_nc():
    if "nc" not in _NC_CACHE:
        _NC_CACHE["nc"] = build_nc()
    return _NC_CACHE["nc"]


def make_in_maps(x, Wq, Wk, Wv, Wo):
    in_maps = []
    for core in range(8):
        b, g = core // 2, core % 2
        sl = slice(g * 256, (g + 1) * 256)
        wqkv = np.stack(
            [Wq[sl, :].T, Wk[sl, :].T, Wv[sl, :].T], axis=1
        )  # (512, 3, 256)
        # paired Wo layout: wo_p[d + 64*(h%2), h//2, c_out]
        wo_l = Wo[:, sl].reshape(C, HL, D)          # (c_out, h, d)
        wo_p = np.empty((P, 2, C), dtype=np.float32)
        for h in range(HL):
            wo_p[(h % 2) * D:(h % 2) * D + D, h // 2, :] = wo_l[:, h, :].T
        in_maps.append({
            "x": np.ascontiguousarray(x[b]).astype(np.float16),
            "wqkv_t": np.ascontiguousarray(wqkv).astype(np.float16),
            "wo_p": np.ascontiguousarray(wo_p).astype(np.float16),
        })
    return in_maps


LAST_RESULTS = {}


def kernel(x, Wq, Wk, Wv, Wo, _trace=False):
    x = np.asarray(x, dtype=np.float32)
    Wq = np.asarray(Wq, dtype=np.float32)
    Wk = np.asarray(Wk, dtype=np.float32)
    Wv = np.asarray(Wv, dtype=np.float32)
    Wo = np.asarray(Wo, dtype=np.float32)

    nc = get_nc()
    in_maps = make_in_maps(x, Wq, Wk, Wv, Wo)
    res = run_bass_kernel_spmd(
        nc, in_maps, core_ids=list(range(8)), trace=_trace
    )
    LAST_RESULTS["res"] = res
    parts = [np.asarray(r["out_p"]).astype(np.float32) for r in res.results]
    out = np.stack([parts[2 * b] + parts[2 * b + 1] for b in range(4)])
    return out
